# revision 38
# baseline (speedup 1.0000x reference)
"""MiniBatchDiscrimination kernel for 8 Trainium2 NeuronCores.

Reference computation (N=256 samples, A=2048 in_features, B=64 out_features,
C=32 kernel dim):
    M  = (f @ T).reshape(N, B, C)
    L1[i,j,b] = sum_c |M[j,b,c] - M[i,b,c]|
    o[j,b]    = sum_i exp(-L1[i,j,b])        (includes the i==j self term = 1)
    out = concat([f, o], axis=1)

Strategy (retrieval-knn pruning): ||v||_1 >= ||v||_2, so the squared-L2
screen D2[i,j,b] = n_i + n_j - 2*G[i,j,b] >= T_SCREEN certifies every
dropped pair contributes < 3e-15 to o.  For this input class the only
survivors are the diagonal (o == 1 exactly, matching fp32 reference).
The host verifies (any accum != clean value => exact recompute of the
affected feature columns), so the result is correct for ALL inputs.

Sharding: tensor-parallel over the B*C columns of T: core d computes
o[:, 8d:8d+8] with no collectives.

v4 device pipeline per core (cost-model guided):
  - fp8 inputs, 5 HWDGE DMAs on the SP queue ordered [fT(k0-7), Tbh0(k0-7),
    fT(k8-15), Tbh0(k8-15), Tbh1] so half-0's GEMM streams behind the DMA
    conveyor and half-1 starts as soon as its block lands.
  - GEMM M^T per 128-row half via DoubleRow fp8 matmuls, split into two
    128-column chunks (samples H0 = 0:128, H1 = 128:256) in one PSUM bank.
  - per half: DVE copies chunk A -> bf16 msbA and squares it (ssbA);
    Pool copies chunk B; DVE squares B.
  - triangle screen per feature b (only unordered pairs): three [128,128]
    blocks: r0 = (i in H0) x (j in H0), r1 = (i in H1) x (j in H0),
    r2 = (i in H1) x (j in H1), all in one PSUM bank [128,4,128].
    The n_j fold rides as a K=32 matmul with constant -1/2 lhsT against
    ssb directly (no norm-row extraction needed); n_i enters as a
    per-partition threshold: tiny N=1 matmuls ssb^T @ ones give norm
    columns nT, ACT rescales to (n - T)/2 thresholds in SBUF.
  - indicators: is_gt(G', (n_i-T)/2) with per-partition scalar (DVE/Pool
    tensor_scalar) or Sign((n_i-T)/2 - G') (ACT, bias vector), accum_out
    per partition into o_sb.  16 ops split across DVE/ACT/Pool.
  - output via a prepared SWDGE dma_scatter_add ([128,16] f32 payload)
    fired by trigger_dma; the Tile end-drain's phantom DMASW wait is
    remapped post-compile to the real completion semaphore.
"""

import os

import ml_dtypes
import numpy as np

N = 256  # batch
A = 2048  # in_features
B = 64  # out_features
C = 32  # kernel dim
NCORES = 8
BLOCAL = B // NCORES  # 8 b-features per core
KT = A // 128  # 16 k-tiles
# Squared-L2 screen threshold: measured min off-diagonal computed D2 for
# fp8 f AND fp8 T is 1.64e4, 6.5x above T_SCREEN; identical rows compute
# D2 ~ 1e2 << T.  Computed D2 >= T still implies true L1 >= ~34.
T_SCREEN = 2500.0

_FP8 = ml_dtypes.float8_e4m3
# scatter indices: idx k at (partition k%16, col k//16), replicated over
# the 8 16-partition channel groups
_IDX = np.ascontiguousarray(
    ((np.arange(128)[:, None] % 16) + 16 * np.arange(8)[None, :]).astype(np.int16)
)

# indicator engine per pair (t, ph): pair covers features g = 2ph, 2ph+1
# of half t; one [128, 2, 3, 128] op per pair.  'D' = DVE is_gt (clean
# accum = 4.0: two diagonal hits per bank), 'A' = ACT Sign (clean =
# 764 safe - 4 diag = 760.0).
PAIR_ENG = {(0, 0): "A", (0, 1): "D", (1, 0): "A", (1, 1): "D"}
# o_sb columns: 0..2 = pair accums ((0,0) A, (0,1) D, (1,0) A); the last
# pair is split per bank: col 3 = bank u0 on DVE, col 4 = bank u1 on ACT.
CLEAN_COLS = [
    (0, 760.0, (0, 0)), (1, 4.0, (0, 1)), (2, 760.0, (1, 0)),
    (3, 2.0, (1, 1, 0)), (4, 380.0, (1, 1, 1)),
]


def _clean_val(eng):
    return 4.0 if eng == "D" else 760.0


_compiled = None
last_run_info = None


def _emit_body(nc, mybir, tc, pools):
    f32 = mybir.dt.float32
    bf16 = mybir.dt.bfloat16
    fp8 = mybir.dt.float8e4
    inp, work, indo, pmt, pgb = (
        pools["inp"], pools["work"], pools["indo"],
        pools["pmt"], pools["pgb"],
    )
    p1_d, p2_d, tb1_d, o_d = pools["dram"]
    DR = mybir.MatmulPerfMode.DoubleRow

    # ---- tiny consts (pre-barrier work kept minimal, split across engines)
    negHalf = work.tile([128, 256], bf16, tag="negh")
    nc.vector.memset(negHalf[:], -0.5)
    biasA = work.tile([128, 1], f32, tag="biasA")
    nc.vector.memset(biasA[:], -T_SCREEN / 2.0)
    o_sb = work.tile([128, 1, 64], f32, tag="osb")
    nc.vector.memset(o_sb[:], 0.0)

    # ---- input DMAs, all on the SP HWDGE queue (issue order == transfer
    # order; issues overlap the previous transfer)
    p1t = inp.tile([128, 3072], fp8, tag="p1")
    p2t = inp.tile([128, 3072], fp8, tag="p2")
    tb1s = inp.tile([128, 16, 128], fp8, tag="tb1")
    nc.sync.dma_start(p1t[:], p1_d[:])
    nc.sync.dma_start(p2t[:, 0:1536], p2_d[:, 0:1536])
    nc.sync.dma_start(p2t[:, 1536:3072], p2_d[:, 1536:3072])
    nc.sync.dma_start(tb1s[:], tb1_d[:])
    # pre-zero the scatter-add target (o_sb is still all-zero here)
    nc.sync.dma_start(o_d[:], o_sb[:, 0, :])

    def ft_pair(kk, w):
        if kk < 4:
            v = p1t[:, 512 * kk : 512 * kk + 512]
        else:
            # p2 sub-chunk layout: [ft kk, tb0 kk] x {45, 67}
            k = kk % 4
            v = p2t[:, 1536 * (k // 2) + 512 * (k % 2) : ][:, 0:512]
        v = v.rearrange("p (a b) -> p a b", a=2)
        return v[:, :, 128 * w : 128 * (w + 1)]

    def tb0_pair(kk):
        if kk < 4:
            v = p1t[:, 2048 + 256 * kk : 2048 + 256 * kk + 256]
        else:
            k = kk % 4
            v = p2t[:, 1536 * (k // 2) + 1024 + 256 * (k % 2) : ][:, 0:256]
        return v.rearrange("p (a b) -> p a b", a=2)

    # ---- PE pstate warmup: one tiny matmul starts the ramp clock.
    # Writes a dead region of mtp0; the GEMM re-zeroes the bank via start.
    mtp0 = pmt.tile([128, 4, 128], f32, tag="mtp", bufs=2, name="mtp0")
    nc.tensor.matmul(
        mtp0[:, 3, 0:1], negHalf[0:1, 0:128], negHalf[0:1, 0:1],
        start=True, stop=True, skip_group_check=True,
    )

    # ---- ACT table preload (Square/Sign/Copy share every act set)
    dumm = work.tile([128, 1], bf16, tag="dumm")
    nc.scalar.activation(
        dumm[:], negHalf[:, 0:1], mybir.ActivationFunctionType.Square,
        bias=0.0, scale=1.0,
    )

    # ---- prepared SWDGE scatter of o_sb -> o_d, fired at the end
    idxs = work.tile([128, 8], mybir.dt.int16, tag="idxs")
    nc.gpsimd.memset(idxs[:], 0)
    nc.gpsimd.iota(idxs[0:16, :], [[16, 8]], base=0, channel_multiplier=1)
    dma_sem = nc.alloc_semaphore(name="oscat")
    nc.gpsimd.dma_scatter_add(
        o_d[:],
        o_sb[:],
        idxs[:],
        num_idxs=128,
        num_idxs_reg=128,
        elem_size=64,
        prepare_only=True,
        sem=dma_sem,
    )

    # ---- GEMM: M^T half t, chunk w (sample cols 128w:128w+128), one PSUM
    # bank per half, regions 0 (A) / 1 (B), single accumulation group.
    def emit_gemm(t, mtp=None):
        if mtp is None:
            mtp = pmt.tile([128, 4, 128], f32, tag="mtp", bufs=2, name=f"mtp{t}")
        steps = []  # (w, kk) in emission order
        if t == 0:
            steps += [(0, kk) for kk in range(4)] + [(1, kk) for kk in range(4)]
            steps += [(0, 4), (0, 5), (1, 4), (1, 5), (0, 6), (0, 7), (1, 6), (1, 7)]
        else:
            steps += [(0, kk) for kk in range(8)] + [(1, kk) for kk in range(8)]
        last = steps[-1]
        for (w, kk) in steps:
            if t == 0:
                tb_sl = tb0_pair(kk)
            else:
                tb_sl = tb1s[:, 2 * kk : 2 * kk + 2, :]
            ft_sl = ft_pair(kk, w)
            nc.tensor.matmul(
                mtp[:, w, :],
                tb_sl,
                ft_sl,
                start=(w, kk) == steps[0],
                stop=(w, kk) == last,
                perf_mode=DR,
                skip_group_check=True,
            )
        return mtp

    # ---- per half: msb/ssb extraction.  DVE copies chunk A, ACT copies
    # chunk B (gpsimd cannot touch PSUM); Pool squares from SBUF.
    def emit_msq(t, mtp):
        msb = work.tile([128, 256], bf16, tag=f"msb{t}")
        ssb = work.tile([128, 256], bf16, tag=f"ssb{t}")
        nc.vector.tensor_copy(msb[:, 0:128], mtp[:, 0, :])
        nc.vector.tensor_copy(msb[:, 128:256], mtp[:, 1, :])
        nc.vector.tensor_tensor(
            ssb[:, 0:128], msb[:, 0:128], msb[:, 0:128], mybir.AluOpType.mult
        )
        nc.vector.tensor_tensor(
            ssb[:, 128:256], msb[:, 128:256], msb[:, 128:256], mybir.AluOpType.mult
        )
        return msb, ssb

    # ---- per half: screen matmuls on PE.  Pair tile (2 PSUM banks)
    # holds features g=2ph (bank u=0) and g=2ph+1 (u=1); per bank regions
    # r0 = (i in H0) x (j in H0), r1 = (i in H1) x (j in H0),
    # r2 = (i in H1) x (j in H1).  Both norm folds ride as K=32 matmuls
    # against the constant -1/2 tile, so G'' = G - n_i/2 - n_j/2 and the
    # indicator threshold is the constant -T/2.
    def emit_screen_pe(t, msb, ssb):
        gps = []
        for ph in range(2):
            gp = pgb.tile(
                [128, 2, 4, 128], f32, tag="gbp", bufs=3, name=f"gp{t}{ph}"
            )
            gps.append(gp)
            for u in range(2):
                g = 2 * ph + u
                lo, hi = 32 * g, 32 * g + 32
                msb2 = msb[lo:hi, :].rearrange("p (a b) -> p a b", a=2)
                ssb2 = ssb[lo:hi, :].rearrange("p (a b) -> p a b", a=2)
                neg2 = negHalf[lo:hi, :].rearrange("p (a b) -> p a b", a=2)
                tp = dict(tile_position=(lo, 0), skip_group_check=True)
                # r0 block
                nc.tensor.matmul(
                    gp[:, u, 0, :], msb[lo:hi, 0:128], msb[lo:hi, 0:128],
                    start=True, stop=False, **tp,
                )
                nc.tensor.matmul(
                    gp[:, u, 0, :], negHalf[lo:hi, 0:128], ssb[lo:hi, 0:128],
                    start=False, stop=False, **tp,
                )
                nc.tensor.matmul(
                    gp[:, u, 0, :], ssb[lo:hi, 0:128], negHalf[lo:hi, 0:128],
                    start=False, stop=False, **tp,
                )
                # r1 + r2 blocks in one 256-wide pass each
                nc.tensor.matmul(
                    gp[:, u, 1:3, :], msb[lo:hi, 128:256], msb2,
                    start=False, stop=False, **tp,
                )
                nc.tensor.matmul(
                    gp[:, u, 1:3, :], negHalf[lo:hi, 0:128], ssb2,
                    start=False, stop=False, **tp,
                )
                nc.tensor.matmul(
                    gp[:, u, 1:3, :], ssb[lo:hi, 128:256], neg2,
                    start=False, stop=True, **tp,
                )
        return gps

    def emit_ind_op(eng, src_ap, col, io):
        if eng == "A":
            nc.scalar.activation(
                io, src_ap, mybir.ActivationFunctionType.Sign,
                bias=biasA[:, 0:1], scale=-1.0,
                accum_out=o_sb[:, 0, col : col + 1],
            )
        else:
            nc.vector.tensor_scalar(
                io, src_ap, -T_SCREEN / 2.0, None,
                mybir.AluOpType.is_gt, mybir.AluOpType.add,
                accum_out=o_sb[:, 0, col : col + 1],
            )

    def emit_ind(t, ph, gp):
        if (t, ph) == (1, 1):
            # last pair: split per bank across both engines in parallel
            ioD = indo.tile([128, 3, 128], fp8, tag="indD", bufs=1, name="indD")
            ioA = indo.tile([128, 3, 128], fp8, tag="indA", bufs=1, name="indA")
            emit_ind_op("D", gp[:, 0, 0:3, :], 3, ioD[:])
            emit_ind_op("A", gp[:, 1, 0:3, :], 4, ioA[:])
        else:
            io = indo.tile([128, 2, 3, 128], fp8, tag="ind", bufs=2, name="ind")
            emit_ind_op(PAIR_ENG[(t, ph)], gp[:, :, 0:3, :], 2 * t + ph, io[:])

    emit_gemm(0, mtp0)
    mtp1 = emit_gemm(1)
    m0 = emit_msq(0, mtp0)
    gps0 = emit_screen_pe(0, *m0)
    m1 = emit_msq(1, mtp1)
    emit_ind(0, 0, gps0[0])
    emit_ind(0, 1, gps0[1])
    gps1 = emit_screen_pe(1, *m1)
    emit_ind(1, 0, gps1[0])
    emit_ind(1, 1, gps1[1])

    # fire the prepared scatter; Tile moves the o_sb data deps here
    nc.gpsimd.trigger_dma(count=None)


def _build():
    import concourse.mybir as mybir
    import concourse.tile as tile
    from concourse import bacc

    f32 = mybir.dt.float32
    fp8 = mybir.dt.float8e4

    nc = bacc.Bacc(None, target_bir_lowering=False, debug=False)
    p1_d = nc.dram_tensor("p1", [128, 3072], fp8, kind="ExternalInput")
    p2_d = nc.dram_tensor("p2", [128, 3072], fp8, kind="ExternalInput")
    tb1_d = nc.dram_tensor("tb1", [128, 2048], fp8, kind="ExternalInput")
    o_d = nc.dram_tensor("o", [128, 64], f32, kind="ExternalOutput")

    with tile.TileContext(nc) as tc:
        with (
            tc.tile_pool(name="inp", bufs=1) as inp,
            tc.tile_pool(name="work", bufs=1) as work,
            tc.tile_pool(name="indo", bufs=2) as indo,
            tc.tile_pool(name="pmt", bufs=1, space="PSUM") as pmt,
            tc.tile_pool(name="pgb", bufs=1, space="PSUM") as pgb,
        ):
            pools = {
                "inp": inp, "work": work, "indo": indo,
                "pmt": pmt, "pgb": pgb,
                "dram": (p1_d, p2_d, tb1_d, o_d),
            }
            _emit_body(nc, mybir, tc, pools)

    nc.compile()

    # Tile's end-of-program drain accounts the prepared scatter on the DMASW0
    # lane, but a gen_mode==1 prep signals its completion through the explicit
    # `sem=` (oscat) instead — the DMASW0 wait would deadlock.  Remap those
    # waits to the real completion sem (same +16, same semantics).
    oscat = None
    for inst in nc.inst_map.values():
        si = inst.sync_info
        if si is None:
            continue
        for u in si.on_update:
            if u.ant_name == "oscat":
                oscat = (u.id, u.ant_name)
    assert oscat is not None
    attached = {}
    for inst in nc.inst_map.values():
        si = inst.sync_info
        if si is None:
            continue
        for u in si.on_update:
            attached[u.id] = attached.get(u.id, 0) + (u.update_value or 0)
    for inst in nc.inst_map.values():
        si = inst.sync_info
        if si is None or not si.on_wait:
            continue

        def _phantom(w):
            return (
                w.ant_name
                and w.ant_name.startswith("DMASW")
                and (w.wait_value or 0) > attached.get(w.id, 0)
            )

        if any(_phantom(w) for w in si.on_wait):
            new_waits = [
                mybir.SyncWait(
                    sync_type="semaphore",
                    id=oscat[0],
                    ant_name=oscat[1],
                    wait_mode="sem-ge-imm",
                    wait_value=16,
                    wait_reg=None,
                )
                if _phantom(w)
                else w
                for w in si.on_wait
            ]
            inst.sync_info = mybir.SyncInfo(
                on_wait=new_waits, on_update=list(si.on_update)
            )
    return nc


def _get_compiled():
    global _compiled
    if _compiled is None:
        _compiled = _build()
    return _compiled


def _host_exact_o_column(f64, T64, b):
    """Exact (float64) o[:, b] for one feature column; used only when the
    device screen detects a potential near-duplicate pair."""
    Mb = f64 @ T64[:, C * b : C * (b + 1)]  # (N, C)
    L1 = np.abs(Mb[None, :, :] - Mb[:, None, :]).sum(axis=2)  # (N, N)
    return np.exp(-L1).sum(axis=0)


def _tile_rows(x):
    """(A, W) row-major -> (128, KT*W) partition-major (row p = k-tiles concat)."""
    w = x.shape[1]
    return np.ascontiguousarray(
        x.reshape(KT, 128, w).transpose(1, 0, 2).reshape(128, KT * w)
    )


def make_in_maps(f, T):
    fT = _tile_rows(f.T.astype(_FP8))
    maps = []
    for d in range(NCORES):
        Tb = T[:, 256 * d : 256 * (d + 1)].astype(_FP8)  # (2048, 256)
        # [128p, half, kt, 128cols]
        Tb4 = np.ascontiguousarray(
            Tb.reshape(KT, 128, 2, 128).transpose(1, 2, 0, 3)
        )
        tb0 = Tb4[:, 0].reshape(128, 2048)
        p2 = np.concatenate(
            [
                fT[:, 2048:3072], tb0[:, 1024:1536],
                fT[:, 3072:4096], tb0[:, 1536:2048],
            ],
            axis=1,
        )
        maps.append(
            {
                "p1": np.ascontiguousarray(
                    np.concatenate([fT[:, :2048], tb0[:, :1024]], axis=1)
                ),
                "p2": np.ascontiguousarray(p2),
                "tb1": np.ascontiguousarray(Tb4[:, 1].reshape(128, 2048)),
            }
        )
    return maps


def kernel(f, T):
    from concourse.bass_utils import run_bass_kernel_spmd

    global last_run_info
    f = np.asarray(f)
    T = np.asarray(T)
    assert f.shape == (N, A) and T.shape == (A, B * C), (f.shape, T.shape)

    nc = _get_compiled()
    in_maps = make_in_maps(f, T)
    res = run_bass_kernel_spmd(
        nc,
        in_maps,
        core_ids=list(range(NCORES)),
        trace=bool(int(os.environ.get("KERNEL_TRACE", "0"))),
    )
    last_run_info = res

    # Device returns per pair (t, ph) the per-partition accum; clean inputs
    # give exactly _clean_val everywhere.  Any other value (near-duplicate
    # pair somewhere in those two feature columns) => exact host recompute.
    o = np.ones((N, B), dtype=np.float32)
    bad = []
    for d in range(NCORES):
        od = np.array(res.results[d]["o"])  # [128, 64]
        for col, cv, key in CLEAN_COLS:
            if np.any(od[:, col] != cv):
                t, ph = key[0], key[1]
                if len(key) == 2:
                    bad.append(BLOCAL * d + 4 * t + 2 * ph)
                    bad.append(BLOCAL * d + 4 * t + 2 * ph + 1)
                else:
                    bad.append(BLOCAL * d + 4 * t + 2 * ph + key[2])
    if bad:
        f64 = f.astype(np.float64)
        T64 = T.astype(np.float64)
        for b in sorted(set(bad)):
            o[:, b] = _host_exact_o_column(f64, T64, int(b)).astype(np.float32)

    return np.concatenate([f.astype(np.float32, copy=False), o], axis=1)


# revision 39
# speedup vs baseline: 1.0259x; 1.0259x over previous
"""MiniBatchDiscrimination kernel for 8 Trainium2 NeuronCores.

Reference computation (N=256 samples, A=2048 in_features, B=64 out_features,
C=32 kernel dim):
    M  = (f @ T).reshape(N, B, C)
    L1[i,j,b] = sum_c |M[j,b,c] - M[i,b,c]|
    o[j,b]    = sum_i exp(-L1[i,j,b])        (includes the i==j self term = 1)
    out = concat([f, o], axis=1)

Strategy (retrieval-knn pruning): ||v||_1 >= ||v||_2, so the squared-L2
screen D2[i,j,b] = n_i + n_j - 2*G[i,j,b] >= T_SCREEN certifies every
dropped pair contributes < 3e-15 to o.  For this input class the only
survivors are the diagonal (o == 1 exactly, matching fp32 reference).
The host verifies (any accum != clean value => exact recompute of the
affected feature columns), so the result is correct for ALL inputs.

Sharding: tensor-parallel over the B*C columns of T: core d computes
o[:, 8d:8d+8] with no collectives.

v4 device pipeline per core (cost-model guided):
  - fp8 inputs, 5 HWDGE DMAs on the SP queue ordered [fT(k0-7), Tbh0(k0-7),
    fT(k8-15), Tbh0(k8-15), Tbh1] so half-0's GEMM streams behind the DMA
    conveyor and half-1 starts as soon as its block lands.
  - GEMM M^T per 128-row half via DoubleRow fp8 matmuls, split into two
    128-column chunks (samples H0 = 0:128, H1 = 128:256) in one PSUM bank.
  - per half: DVE copies chunk A -> bf16 msbA and squares it (ssbA);
    Pool copies chunk B; DVE squares B.
  - triangle screen per feature b (only unordered pairs): three [128,128]
    blocks: r0 = (i in H0) x (j in H0), r1 = (i in H1) x (j in H0),
    r2 = (i in H1) x (j in H1), all in one PSUM bank [128,4,128].
    The n_j fold rides as a K=32 matmul with constant -1/2 lhsT against
    ssb directly (no norm-row extraction needed); n_i enters as a
    per-partition threshold: tiny N=1 matmuls ssb^T @ ones give norm
    columns nT, ACT rescales to (n - T)/2 thresholds in SBUF.
  - indicators: is_gt(G', (n_i-T)/2) with per-partition scalar (DVE/Pool
    tensor_scalar) or Sign((n_i-T)/2 - G') (ACT, bias vector), accum_out
    per partition into o_sb.  16 ops split across DVE/ACT/Pool.
  - output via a prepared SWDGE dma_scatter_add ([128,16] f32 payload)
    fired by trigger_dma; the Tile end-drain's phantom DMASW wait is
    remapped post-compile to the real completion semaphore.
"""

import os

import ml_dtypes
import numpy as np

N = 256  # batch
A = 2048  # in_features
B = 64  # out_features
C = 32  # kernel dim
NCORES = 8
BLOCAL = B // NCORES  # 8 b-features per core
KT = A // 128  # 16 k-tiles
# Squared-L2 screen threshold: measured min off-diagonal computed D2 for
# fp8 f AND fp8 T is 1.64e4, 6.5x above T_SCREEN; identical rows compute
# D2 ~ 1e2 << T.  Computed D2 >= T still implies true L1 >= ~34.
T_SCREEN = 2500.0

_FP8 = ml_dtypes.float8_e4m3
# scatter indices: idx k at (partition k%16, col k//16), replicated over
# the 8 16-partition channel groups
_IDX = np.ascontiguousarray(
    ((np.arange(128)[:, None] % 16) + 16 * np.arange(8)[None, :]).astype(np.int16)
)

# indicator engine per pair (t, ph): pair covers features g = 2ph, 2ph+1
# of half t; one [128, 2, 3, 128] op per pair.  'D' = DVE is_gt (clean
# accum = 4.0: two diagonal hits per bank), 'A' = ACT Sign (clean =
# 764 safe - 4 diag = 760.0).
PAIR_ENG = {(0, 0): "A", (0, 1): "D", (1, 0): "A", (1, 1): "D"}
# o_sb columns: 0..2 = pair accums ((0,0) A, (0,1) D, (1,0) A); the last
# pair is split per bank: col 3 = bank u0 on DVE, col 4 = bank u1 on ACT.
CLEAN_COLS = [
    (0, 760.0, (0, 0)), (1, 4.0, (0, 1)), (2, 760.0, (1, 0)),
    (3, 4.0, (1, 1)),
]


def _clean_val(eng):
    return 4.0 if eng == "D" else 760.0


_compiled = None
last_run_info = None


def _emit_body(nc, mybir, tc, pools):
    f32 = mybir.dt.float32
    bf16 = mybir.dt.bfloat16
    fp8 = mybir.dt.float8e4
    inp, work, indo, pmt, pgb = (
        pools["inp"], pools["work"], pools["indo"],
        pools["pmt"], pools["pgb"],
    )
    p1_d, p2_d, tb1_d, o_d = pools["dram"]
    DR = mybir.MatmulPerfMode.DoubleRow

    # ---- tiny consts (pre-barrier work kept minimal, split across engines)
    negHalf = work.tile([128, 256], bf16, tag="negh")
    nc.vector.memset(negHalf[:], -0.5)
    biasA = work.tile([128, 1], f32, tag="biasA")
    nc.vector.memset(biasA[:], -T_SCREEN / 2.0)
    o_sb = work.tile([128, 1, 64], f32, tag="osb")
    nc.vector.memset(o_sb[:], 0.0)

    # ---- input DMAs, all on the SP HWDGE queue (issue order == transfer
    # order; issues overlap the previous transfer)
    p1t = inp.tile([128, 3072], fp8, tag="p1")
    p2t = inp.tile([128, 3072], fp8, tag="p2")
    tb1s = inp.tile([128, 16, 128], fp8, tag="tb1")
    nc.sync.dma_start(p1t[:], p1_d[:])
    nc.sync.dma_start(p2t[:, 0:1536], p2_d[:, 0:1536])
    nc.sync.dma_start(p2t[:, 1536:3072], p2_d[:, 1536:3072])
    nc.sync.dma_start(tb1s[:], tb1_d[:])
    # pre-zero the scatter-add target (o_sb is still all-zero here)
    nc.sync.dma_start(o_d[:], o_sb[:, 0, :])

    def ft_pair(kk, w):
        if kk < 4:
            v = p1t[:, 512 * kk : 512 * kk + 512]
        else:
            # p2 sub-chunk layout: [ft kk, tb0 kk] x {45, 67}
            k = kk % 4
            v = p2t[:, 1536 * (k // 2) + 512 * (k % 2) : ][:, 0:512]
        v = v.rearrange("p (a b) -> p a b", a=2)
        return v[:, :, 128 * w : 128 * (w + 1)]

    def tb0_pair(kk):
        if kk < 4:
            v = p1t[:, 2048 + 256 * kk : 2048 + 256 * kk + 256]
        else:
            k = kk % 4
            v = p2t[:, 1536 * (k // 2) + 1024 + 256 * (k % 2) : ][:, 0:256]
        return v.rearrange("p (a b) -> p a b", a=2)

    # ---- PE pstate warmup: one tiny matmul starts the ramp clock.
    # Writes a dead region of mtp0; the GEMM re-zeroes the bank via start.
    mtp0 = pmt.tile([128, 4, 128], f32, tag="mtp", bufs=2, name="mtp0")
    nc.tensor.matmul(
        mtp0[:, 3, 0:1], negHalf[0:1, 0:128], negHalf[0:1, 0:1],
        start=True, stop=True, skip_group_check=True,
    )

    # ---- ACT table preload (Square/Sign/Copy share every act set)
    dumm = work.tile([128, 1], bf16, tag="dumm")
    nc.scalar.activation(
        dumm[:], negHalf[:, 0:1], mybir.ActivationFunctionType.Square,
        bias=0.0, scale=1.0,
    )

    # ---- prepared SWDGE scatter of o_sb -> o_d, fired at the end
    idxs = work.tile([128, 8], mybir.dt.int16, tag="idxs")
    nc.gpsimd.memset(idxs[:], 0)
    nc.gpsimd.iota(idxs[0:16, :], [[16, 8]], base=0, channel_multiplier=1)
    dma_sem = nc.alloc_semaphore(name="oscat")
    nc.gpsimd.dma_scatter_add(
        o_d[:],
        o_sb[:],
        idxs[:],
        num_idxs=128,
        num_idxs_reg=128,
        elem_size=64,
        prepare_only=True,
        sem=dma_sem,
    )

    # ---- GEMM: M^T half t, chunk w (sample cols 128w:128w+128), one PSUM
    # bank per half, regions 0 (A) / 1 (B), single accumulation group.
    def emit_gemm(t, mtp=None):
        if mtp is None:
            mtp = pmt.tile([128, 4, 128], f32, tag="mtp", bufs=2, name=f"mtp{t}")
        steps = []  # (w, kk) in emission order
        if t == 0:
            steps += [(0, kk) for kk in range(4)] + [(1, kk) for kk in range(4)]
            steps += [(0, 4), (0, 5), (1, 4), (1, 5), (0, 6), (0, 7), (1, 6), (1, 7)]
        else:
            steps += [(0, kk) for kk in range(8)] + [(1, kk) for kk in range(8)]
        last = steps[-1]
        for (w, kk) in steps:
            if t == 0:
                tb_sl = tb0_pair(kk)
            else:
                tb_sl = tb1s[:, 2 * kk : 2 * kk + 2, :]
            ft_sl = ft_pair(kk, w)
            nc.tensor.matmul(
                mtp[:, w, :],
                tb_sl,
                ft_sl,
                start=(w, kk) == steps[0],
                stop=(w, kk) == last,
                perf_mode=DR,
                skip_group_check=True,
            )
        return mtp

    # ---- per half: msb/ssb extraction.  DVE copies chunk A, ACT copies
    # chunk B (gpsimd cannot touch PSUM); Pool squares from SBUF.
    def emit_msq(t, mtp):
        msb = work.tile([128, 256], bf16, tag=f"msb{t}")
        ssb = work.tile([128, 256], bf16, tag=f"ssb{t}")
        nc.vector.tensor_copy(msb[:, 0:128], mtp[:, 0, :])
        nc.vector.tensor_copy(msb[:, 128:256], mtp[:, 1, :])
        nc.vector.tensor_tensor(
            ssb[:, 0:128], msb[:, 0:128], msb[:, 0:128], mybir.AluOpType.mult
        )
        nc.vector.tensor_tensor(
            ssb[:, 128:256], msb[:, 128:256], msb[:, 128:256], mybir.AluOpType.mult
        )
        return msb, ssb

    # ---- per half: screen matmuls on PE.  Pair tile (2 PSUM banks)
    # holds features g=2ph (bank u=0) and g=2ph+1 (u=1); per bank regions
    # r0 = (i in H0) x (j in H0), r1 = (i in H1) x (j in H0),
    # r2 = (i in H1) x (j in H1).  Both norm folds ride as K=32 matmuls
    # against the constant -1/2 tile, so G'' = G - n_i/2 - n_j/2 and the
    # indicator threshold is the constant -T/2.
    def emit_screen_pe(t, msb, ssb):
        gps = []
        for ph in range(2):
            gp = pgb.tile(
                [128, 2, 4, 128], f32, tag="gbp", bufs=3, name=f"gp{t}{ph}"
            )
            gps.append(gp)
            for u in range(2):
                g = 2 * ph + u
                lo, hi = 32 * g, 32 * g + 32
                msb2 = msb[lo:hi, :].rearrange("p (a b) -> p a b", a=2)
                ssb2 = ssb[lo:hi, :].rearrange("p (a b) -> p a b", a=2)
                neg2 = negHalf[lo:hi, :].rearrange("p (a b) -> p a b", a=2)
                tp = dict(tile_position=(lo, 0), skip_group_check=True)
                # r0 block
                nc.tensor.matmul(
                    gp[:, u, 0, :], msb[lo:hi, 0:128], msb[lo:hi, 0:128],
                    start=True, stop=False, **tp,
                )
                nc.tensor.matmul(
                    gp[:, u, 0, :], negHalf[lo:hi, 0:128], ssb[lo:hi, 0:128],
                    start=False, stop=False, **tp,
                )
                nc.tensor.matmul(
                    gp[:, u, 0, :], ssb[lo:hi, 0:128], negHalf[lo:hi, 0:128],
                    start=False, stop=False, **tp,
                )
                # r1 + r2 blocks in one 256-wide pass each
                nc.tensor.matmul(
                    gp[:, u, 1:3, :], msb[lo:hi, 128:256], msb2,
                    start=False, stop=False, **tp,
                )
                nc.tensor.matmul(
                    gp[:, u, 1:3, :], negHalf[lo:hi, 0:128], ssb2,
                    start=False, stop=False, **tp,
                )
                nc.tensor.matmul(
                    gp[:, u, 1:3, :], ssb[lo:hi, 128:256], neg2,
                    start=False, stop=True, **tp,
                )
        return gps

    def emit_ind_op(eng, src_ap, col, io):
        if eng == "A":
            nc.scalar.activation(
                io, src_ap, mybir.ActivationFunctionType.Sign,
                bias=biasA[:, 0:1], scale=-1.0,
                accum_out=o_sb[:, 0, col : col + 1],
            )
        else:
            nc.vector.tensor_scalar(
                io, src_ap, -T_SCREEN / 2.0, None,
                mybir.AluOpType.is_gt, mybir.AluOpType.add,
                accum_out=o_sb[:, 0, col : col + 1],
            )

    def emit_ind(t, ph, gp):
        io = indo.tile([128, 2, 3, 128], fp8, tag="ind", bufs=4, name="ind")
        emit_ind_op(PAIR_ENG[(t, ph)], gp[:, :, 0:3, :], 2 * t + ph, io[:])

    emit_gemm(0, mtp0)
    mtp1 = emit_gemm(1)
    m0 = emit_msq(0, mtp0)
    gps0 = emit_screen_pe(0, *m0)
    m1 = emit_msq(1, mtp1)
    emit_ind(0, 0, gps0[0])
    emit_ind(0, 1, gps0[1])
    gps1 = emit_screen_pe(1, *m1)
    emit_ind(1, 0, gps1[0])
    emit_ind(1, 1, gps1[1])

    # fire the prepared scatter; Tile moves the o_sb data deps here
    nc.gpsimd.trigger_dma(count=None)


def _build():
    import concourse.mybir as mybir
    import concourse.tile as tile
    from concourse import bacc

    f32 = mybir.dt.float32
    fp8 = mybir.dt.float8e4

    nc = bacc.Bacc(None, target_bir_lowering=False, debug=False)
    p1_d = nc.dram_tensor("p1", [128, 3072], fp8, kind="ExternalInput")
    p2_d = nc.dram_tensor("p2", [128, 3072], fp8, kind="ExternalInput")
    tb1_d = nc.dram_tensor("tb1", [128, 2048], fp8, kind="ExternalInput")
    o_d = nc.dram_tensor("o", [128, 64], f32, kind="ExternalOutput")

    with tile.TileContext(nc) as tc:
        with (
            tc.tile_pool(name="inp", bufs=1) as inp,
            tc.tile_pool(name="work", bufs=1) as work,
            tc.tile_pool(name="indo", bufs=2) as indo,
            tc.tile_pool(name="pmt", bufs=1, space="PSUM") as pmt,
            tc.tile_pool(name="pgb", bufs=1, space="PSUM") as pgb,
        ):
            pools = {
                "inp": inp, "work": work, "indo": indo,
                "pmt": pmt, "pgb": pgb,
                "dram": (p1_d, p2_d, tb1_d, o_d),
            }
            _emit_body(nc, mybir, tc, pools)

    nc.compile()

    # Tile's end-of-program drain accounts the prepared scatter on the DMASW0
    # lane, but a gen_mode==1 prep signals its completion through the explicit
    # `sem=` (oscat) instead — the DMASW0 wait would deadlock.  Remap those
    # waits to the real completion sem (same +16, same semantics).
    oscat = None
    for inst in nc.inst_map.values():
        si = inst.sync_info
        if si is None:
            continue
        for u in si.on_update:
            if u.ant_name == "oscat":
                oscat = (u.id, u.ant_name)
    assert oscat is not None
    attached = {}
    for inst in nc.inst_map.values():
        si = inst.sync_info
        if si is None:
            continue
        for u in si.on_update:
            attached[u.id] = attached.get(u.id, 0) + (u.update_value or 0)
    for inst in nc.inst_map.values():
        si = inst.sync_info
        if si is None or not si.on_wait:
            continue

        def _phantom(w):
            return (
                w.ant_name
                and w.ant_name.startswith("DMASW")
                and (w.wait_value or 0) > attached.get(w.id, 0)
            )

        if any(_phantom(w) for w in si.on_wait):
            new_waits = [
                mybir.SyncWait(
                    sync_type="semaphore",
                    id=oscat[0],
                    ant_name=oscat[1],
                    wait_mode="sem-ge-imm",
                    wait_value=16,
                    wait_reg=None,
                )
                if _phantom(w)
                else w
                for w in si.on_wait
            ]
            inst.sync_info = mybir.SyncInfo(
                on_wait=new_waits, on_update=list(si.on_update)
            )
    return nc


def _get_compiled():
    global _compiled
    if _compiled is None:
        _compiled = _build()
    return _compiled


def _host_exact_o_column(f64, T64, b):
    """Exact (float64) o[:, b] for one feature column; used only when the
    device screen detects a potential near-duplicate pair."""
    Mb = f64 @ T64[:, C * b : C * (b + 1)]  # (N, C)
    L1 = np.abs(Mb[None, :, :] - Mb[:, None, :]).sum(axis=2)  # (N, N)
    return np.exp(-L1).sum(axis=0)


def _tile_rows(x):
    """(A, W) row-major -> (128, KT*W) partition-major (row p = k-tiles concat)."""
    w = x.shape[1]
    return np.ascontiguousarray(
        x.reshape(KT, 128, w).transpose(1, 0, 2).reshape(128, KT * w)
    )


def make_in_maps(f, T):
    fT = _tile_rows(f.T.astype(_FP8))
    maps = []
    for d in range(NCORES):
        Tb = T[:, 256 * d : 256 * (d + 1)].astype(_FP8)  # (2048, 256)
        # [128p, half, kt, 128cols]
        Tb4 = np.ascontiguousarray(
            Tb.reshape(KT, 128, 2, 128).transpose(1, 2, 0, 3)
        )
        tb0 = Tb4[:, 0].reshape(128, 2048)
        p2 = np.concatenate(
            [
                fT[:, 2048:3072], tb0[:, 1024:1536],
                fT[:, 3072:4096], tb0[:, 1536:2048],
            ],
            axis=1,
        )
        maps.append(
            {
                "p1": np.ascontiguousarray(
                    np.concatenate([fT[:, :2048], tb0[:, :1024]], axis=1)
                ),
                "p2": np.ascontiguousarray(p2),
                "tb1": np.ascontiguousarray(Tb4[:, 1].reshape(128, 2048)),
            }
        )
    return maps


def kernel(f, T):
    from concourse.bass_utils import run_bass_kernel_spmd

    global last_run_info
    f = np.asarray(f)
    T = np.asarray(T)
    assert f.shape == (N, A) and T.shape == (A, B * C), (f.shape, T.shape)

    nc = _get_compiled()
    in_maps = make_in_maps(f, T)
    res = run_bass_kernel_spmd(
        nc,
        in_maps,
        core_ids=list(range(NCORES)),
        trace=bool(int(os.environ.get("KERNEL_TRACE", "0"))),
    )
    last_run_info = res

    # Device returns per pair (t, ph) the per-partition accum; clean inputs
    # give exactly _clean_val everywhere.  Any other value (near-duplicate
    # pair somewhere in those two feature columns) => exact host recompute.
    o = np.ones((N, B), dtype=np.float32)
    bad = []
    for d in range(NCORES):
        od = np.array(res.results[d]["o"])  # [128, 64]
        for col, cv, key in CLEAN_COLS:
            if np.any(od[:, col] != cv):
                t, ph = key[0], key[1]
                if len(key) == 2:
                    bad.append(BLOCAL * d + 4 * t + 2 * ph)
                    bad.append(BLOCAL * d + 4 * t + 2 * ph + 1)
                else:
                    bad.append(BLOCAL * d + 4 * t + 2 * ph + key[2])
    if bad:
        f64 = f.astype(np.float64)
        T64 = T.astype(np.float64)
        for b in sorted(set(bad)):
            o[:, b] = _host_exact_o_column(f64, T64, int(b)).astype(np.float32)

    return np.concatenate([f.astype(np.float32, copy=False), o], axis=1)
